# revision 19
# baseline (speedup 1.0000x reference)
"""Trainium2 Bass kernel for nn_Attention_10797547782838.

Windowed multi-head attention with per-query angle bias:
  q = (x@Wq+bq) reshaped to heads; k,v = x@Wkv+bkv
  attn = (q*scale) @ k^T * anglebias(q) + mask[b%4]; softmax; @v; proj Wp.

Sharding: batch (16) data-parallel over 8 cores, 2 batches/core.
Device layout: S^T (keys on partitions, queries free). Angle bias and
hd^-0.5 are folded into q at projection time; the window mask is applied
multiplicatively after exp (P = exp(S^T) * exp(mask)^T); softmax
denominators come from ones-column matmuls; normalization is applied to
O^T via a selector-matmul broadcast of 1/rowsum before the output
projection.
"""
import math
import sys

import numpy as np

sys.path.insert(0, "/opt/trn_rl_repo")

import concourse.bass as bass  # noqa: E402
import concourse.bacc as bacc  # noqa: E402
import concourse.tile as tile  # noqa: E402
from concourse import mybir  # noqa: E402
from concourse.bass_utils import run_bass_kernel_spmd  # noqa: E402

F32 = mybir.dt.float32
F32R = mybir.dt.float32r
BF16 = mybir.dt.bfloat16
AF = mybir.ActivationFunctionType
ALU = mybir.AluOpType

B, N, C = 16, 1024, 256
HEADS, HD = 8, 32
NW = 4
N_CORES = 8
BPC = B // N_CORES  # batches per core

_CACHE = {}


def r32(ap):
    return ap.bitcast(F32R)


def build_kernel():
    from contextlib import ExitStack
    nc = bacc.Bacc("TRN2", target_bir_lowering=False, debug=False,
                   num_devices=N_CORES)

    d_xT = nc.dram_tensor("xT", [BPC, C, N], F32R, kind="ExternalInput").ap()
    d_biasT = nc.dram_tensor("biasT", [BPC, C, N], F32, kind="ExternalInput").ap()
    d_emT = nc.dram_tensor("emT", [BPC, N, N], BF16, kind="ExternalInput").ap()
    d_wq = nc.dram_tensor("wq", [C, C], F32R, kind="ExternalInput").ap()
    d_wk = nc.dram_tensor("wk", [C, C], F32R, kind="ExternalInput").ap()
    d_wv = nc.dram_tensor("wv", [C, C], F32R, kind="ExternalInput").ap()
    d_wp = nc.dram_tensor("wp", [C, C], F32R, kind="ExternalInput").ap()
    d_bq = nc.dram_tensor("bq", [128, 2], F32, kind="ExternalInput").ap()
    d_bk = nc.dram_tensor("bk", [128, 2], F32, kind="ExternalInput").ap()
    d_bv = nc.dram_tensor("bv", [128, C], F32, kind="ExternalInput").ap()
    d_bp = nc.dram_tensor("bp", [128, C], F32, kind="ExternalInput").ap()
    d_y = nc.dram_tensor("y", [BPC, N, C], F32, kind="ExternalOutput").ap()

    with tile.TileContext(nc) as tc:
        with ExitStack() as ctx, nc.allow_low_precision(reason="fp32r matmul inputs; accumulation stays fp32 in PSUM"):
            kernel_body(ctx, tc, d_xT, d_biasT, d_emT, d_wq, d_wk, d_wv,
                        d_wp, d_bq, d_bk, d_bv, d_bp, d_y)
    nc.compile()
    return nc


def kernel_body(ctx, tc, d_xT, d_biasT, d_emT, d_wq, d_wk, d_wv, d_wp,
                d_bq, d_bk, d_bv, d_bp, d_y):
    nc = tc.nc

    consts = ctx.enter_context(tc.tile_pool(name="consts", bufs=1))
    xpool = ctx.enter_context(tc.tile_pool(name="xpool", bufs=2))
    qkv = ctx.enter_context(tc.tile_pool(name="qkv", bufs=1))
    empool = ctx.enter_context(tc.tile_pool(name="empool", bufs=2))
    ppool = ctx.enter_context(tc.tile_pool(name="ppool", bufs=4))
    otpool = ctx.enter_context(tc.tile_pool(name="otpool", bufs=2))
    misc = ctx.enter_context(tc.tile_pool(name="misc", bufs=2))
    ypool = ctx.enter_context(tc.tile_pool(name="ypool", bufs=3))
    psS = ctx.enter_context(tc.tile_pool(name="psS", bufs=2, space="PSUM"))
    psO = ctx.enter_context(tc.tile_pool(name="psO", bufs=1, space="PSUM"))

    # ---- constants / weights (once) ----
    w_sb = {}
    for name, dram in (("wq", d_wq), ("wk", d_wk), ("wv", d_wv), ("wp", d_wp)):
        t = consts.tile([128, 2, C], F32R, tag=f"w_{name}")
        nc.sync.dma_start(out=t[:], in_=dram.rearrange("(c p) n -> p c n", p=128))
        w_sb[name] = t
    bq_sb = consts.tile([128, 2], F32, tag="bq")
    nc.sync.dma_start(out=bq_sb[:], in_=d_bq[:])
    bk_sb = consts.tile([128, 2], F32, tag="bk")
    nc.sync.dma_start(out=bk_sb[:], in_=d_bk[:])
    bv_sb = consts.tile([128, C], F32, tag="bv")
    nc.sync.dma_start(out=bv_sb[:], in_=d_bv[:])
    bp_sb = consts.tile([128, C], F32, tag="bp")
    nc.sync.dma_start(out=bp_sb[:], in_=d_bp[:])
    ones_bf = consts.tile([128, 256], BF16, tag="ones_bf")
    nc.vector.memset(ones_bf[:], 1.0)

    n_dve = 0
    n_gps = 0

    v_sb = [qkv.tile([128, 8, 64], BF16, tag=f"v{t8}", name=f"v{t8}")
            for t8 in range(8)]

    for b in range(BPC):
        # ---- stage inputs for this batch ----
        xT_sb = xpool.tile([128, 2, N], F32R, tag="xT")
        nc.sync.dma_start(out=xT_sb[:], in_=d_xT[b].rearrange("(c p) n -> p c n", p=128))
        biasT_sb = xpool.tile([128, 2, N], F32, tag="biasT")
        nc.sync.dma_start(out=biasT_sb[:], in_=d_biasT[b].rearrange("(c p) n -> p c n", p=128))
        em_sb = [empool.tile([128, N], BF16, tag=f"em{kc}", name=f"em{kc}") for kc in range(8)]
        for kc in range(8):
            nc.sync.dma_start(out=em_sb[kc][:], in_=d_emT[b, kc * 128:(kc + 1) * 128, :])

        # ---- projections ----
        # qT/kT: [c_out chunk m 128, q 1024]
        qT_sb = qkv.tile([128, 2, N], F32R, tag="qT")
        kT_sb = qkv.tile([128, 2, N], F32R, tag="kT")
        for m in range(2):
            ps_q = psS.tile([128, N], F32, tag="s")
            for qc in range(2):
                for ci in range(2):
                    nc.tensor.matmul(
                        ps_q[:, qc * 512:(qc + 1) * 512],
                        w_sb["wq"][:, ci, m * 128:(m + 1) * 128],
                        xT_sb[:, ci, qc * 512:(qc + 1) * 512],
                        start=(ci == 0), stop=(ci == 1))
            # qT = (psum + bq) * biasT
            nc.vector.scalar_tensor_tensor(
                out=qT_sb[:, m, :], in0=ps_q[:], scalar=bq_sb[:, m:m + 1],
                in1=biasT_sb[:, m, :], op0=ALU.add, op1=ALU.mult)
            ps_k = psS.tile([128, N], F32, tag="s")
            for qc in range(2):
                for ci in range(2):
                    nc.tensor.matmul(
                        ps_k[:, qc * 512:(qc + 1) * 512],
                        w_sb["wk"][:, ci, m * 128:(m + 1) * 128],
                        xT_sb[:, ci, qc * 512:(qc + 1) * 512],
                        start=(ci == 0), stop=(ci == 1))
            nc.vector.tensor_scalar_add(
                out=kT_sb[:, m, :], in0=ps_k[:], scalar1=bk_sb[:, m:m + 1])

        # V: aug [128 tok, 8 heads, 64] bf16 = [V_h | ones32]
        if b == 0:
            for t8 in range(8):
                nc.vector.tensor_copy(
                    out=v_sb[t8][:, :, 32:64],
                    in_=ones_bf[:].rearrange("p (h e) -> p h e", h=8))
        for t8 in range(8):
            ps_v = psS.tile([128, C], F32, tag="s")
            for ci in range(2):
                nc.tensor.matmul(
                    ps_v[:],
                    xT_sb[:, ci, t8 * 128:(t8 + 1) * 128],
                    w_sb["wv"][:, ci, :],
                    start=(ci == 0), stop=(ci == 1))
            nc.vector.tensor_tensor(
                out=v_sb[t8][:, :, 0:32],
                in0=ps_v[:].rearrange("p (h e) -> p h e", h=8),
                in1=bv_sb[:].rearrange("p (h e) -> p h e", h=8), op=ALU.add)

        # ---- attention ----
        otn_sb = [otpool.tile([128, N], F32R, tag=f"otn{cc}", name=f"otn{cc}") for cc in range(2)]
        for qc in range(2):
            for hg in range(2):  # head group: heads 4hg..4hg+3
                po = [psO.tile([64, 512], F32, tag=f"po{j}", name=f"po{j}")
                      for j in range(4)]
                for kc in range(8):
                    p_pair = []
                    for pair in range(2):  # heads 4hg+2pair, +1
                        ps_s = psS.tile([128, 1024], F32, tag="s")
                        for hh in range(2):
                            h = 4 * hg + 2 * pair + hh
                            j = h % 4
                            nc.tensor.matmul(
                                ps_s[:, hh * 512:(hh + 1) * 512],
                                kT_sb[32 * j:32 * (j + 1), hg,
                                          kc * 128:(kc + 1) * 128],
                                qT_sb[32 * j:32 * (j + 1), hg,
                                          qc * 512:(qc + 1) * 512],
                                start=True, stop=True,
                                tile_position=(32 * j, 0))
                        p0 = ppool.tile([128, 1024], BF16, tag="p0")
                        nc.scalar.activation(out=p0[:], in_=ps_s[:], func=AF.Exp)
                        # P = P0 * em (mask), split DVE / gpsimd
                        pt = ppool.tile([128, 1024], BF16, tag="pt")
                        emsl = em_sb[kc][:, qc * 512:(qc + 1) * 512]
                        for hh in range(2):
                            use_dve = (n_dve * 26) <= (n_gps * 74)
                            eng = nc.vector if use_dve else nc.gpsimd
                            if use_dve:
                                n_dve += 1
                            else:
                                n_gps += 1
                            eng.tensor_tensor(
                                out=pt[:, hh * 512:(hh + 1) * 512],
                                in0=p0[:, hh * 512:(hh + 1) * 512],
                                in1=emsl, op=ALU.mult)
                        p_pair.append(pt)
                    for pair in range(2):
                        for hh in range(2):
                            h = 4 * hg + 2 * pair + hh
                            j = h % 4
                            rhs = p_pair[pair][:, hh * 512:(hh + 1) * 512]
                            nc.tensor.matmul(
                                po[j][:],
                                v_sb[kc][:, h, :],
                                rhs, start=(kc == 0), stop=(kc == 7))
                # po[j] rows 0-31 = O^T_h, rows 32-63 = rowsum (x32 dup)
                for j in range(4):
                    rb = misc.tile([32, 512], F32, tag="rb")
                    nc.vector.reciprocal(out=rb[:], in_=po[j][32:64, :])
                    nc.vector.tensor_tensor(
                        out=otn_sb[hg][32 * j:32 * (j + 1),
                                       qc * 512:(qc + 1) * 512],
                        in0=po[j][0:32, :], in1=rb[:], op=ALU.mult)

        # ---- output projection: Y[q,c] = Onorm^T.T @ Wp + bp ----
        for qt in range(8):
            ps_y = psS.tile([128, C], F32, tag="s")
            for ci in range(2):
                nc.tensor.matmul(
                    ps_y[:],
                    otn_sb[ci][:, qt * 128:(qt + 1) * 128],
                    w_sb["wp"][:, ci, :],
                    start=(ci == 0), stop=(ci == 1))
            y_sb = ypool.tile([128, C], F32, tag="y")
            nc.vector.tensor_tensor(out=y_sb[:], in0=ps_y[:], in1=bp_sb[:],
                                    op=ALU.add)
            nc.sync.dma_start(out=d_y[b, qt * 128:(qt + 1) * 128, :], in_=y_sb[:])


def _host_prep(x, mask, affine_matrix, Wq, bq, Wkv, bkv, Wp, bp,
               angle_table, H, W):
    B_, N_, C_ = x.shape
    heads = angle_table.shape[1]
    hd = C_ // heads
    scale = np.float32(hd ** -0.5)
    H = int(H); W = int(W)

    gy, gx = np.meshgrid(np.arange(H, dtype=np.float32),
                         np.arange(W, dtype=np.float32), indexing="ij")
    coords = np.stack([gx.reshape(-1), gy.reshape(-1)], -1).astype(np.float32)
    center = np.array([W / 2.0, H / 2.0], np.float32)
    ego = np.einsum("bij,j->bi", affine_matrix[:, :2, :2], center) \
        + affine_matrix[:, :2, 2]
    rel = coords[None, :, :] - ego[:, None, :]
    ang = np.arctan2(rel[..., 1], rel[..., 0]).astype(np.float32)
    bins = (((ang + np.float32(math.pi)) / np.float32(2.0 * math.pi))
            * (angle_table.shape[0] - 1)).astype(np.int32)
    sig = (1.0 / (1.0 + np.exp(-angle_table[bins]))).astype(np.float32)
    bias = (1.0 + sig).astype(np.float32)                      # (B,N,h)

    biasT = np.repeat(bias.transpose(0, 2, 1) * scale, hd, axis=1)  # (B,C,N)
    biasT = np.ascontiguousarray(biasT, dtype=np.float32)
    xT = np.ascontiguousarray(x.transpose(0, 2, 1), dtype=np.float32)
    import ml_dtypes
    emT = np.ascontiguousarray(
        np.exp(mask).transpose(0, 2, 1).astype(ml_dtypes.bfloat16))  # [k,q]

    Wk = np.ascontiguousarray(Wkv[:, :C_], dtype=np.float32)
    Wv = np.ascontiguousarray(Wkv[:, C_:], dtype=np.float32)
    bq2 = np.ascontiguousarray(bq.reshape(2, 128).T, dtype=np.float32)
    bk2 = np.ascontiguousarray(bkv[:C_].reshape(2, 128).T, dtype=np.float32)
    bv_rep = np.ascontiguousarray(
        np.broadcast_to(bkv[C_:], (128, C_)), dtype=np.float32)
    bp_rep = np.ascontiguousarray(
        np.broadcast_to(bp, (128, C_)), dtype=np.float32)
    return xT, biasT, emT, Wk, Wv, bq2, bk2, bv_rep, bp_rep


def _ensure_ntff_hook():
    import types
    try:
        from antenv import axon_hooks  # noqa: F401
        return
    except ImportError:
        pass
    import antenv
    mod = types.ModuleType("antenv.axon_hooks")
    _h = {"hook": None}
    mod.get_axon_ntff_profile_hook = lambda: _h["hook"]
    mod.set_axon_ntff_profile_hook = lambda hook: _h.__setitem__("hook", hook)
    sys.modules["antenv.axon_hooks"] = mod
    antenv.axon_hooks = mod
    try:
        sys.path.insert(0, "/root/.axon_site/trn_agent_boot")
        import trn_boot
        hook = trn_boot._ntff_profile_via_ctypes("/opt/axon/libaxon_pjrt.so")
        if hook is not None:
            mod.set_axon_ntff_profile_hook(hook)
    except Exception as e:
        print("ntff hook setup failed:", repr(e))


def kernel(x, mask, affine_matrix, Wq, bq, Wkv, bkv, Wp, bp,
           angle_table, H, W, _profile=False):
    if _profile:
        _ensure_ntff_hook()
    x = np.asarray(x, np.float32)
    mask = np.asarray(mask, np.float32)
    affine_matrix = np.asarray(affine_matrix, np.float32)
    Wq = np.asarray(Wq, np.float32); bq = np.asarray(bq, np.float32)
    Wkv = np.asarray(Wkv, np.float32); bkv = np.asarray(bkv, np.float32)
    Wp = np.asarray(Wp, np.float32); bp = np.asarray(bp, np.float32)
    angle_table = np.asarray(angle_table, np.float32)

    xT, biasT, emT, Wk, Wv, bq2, bk2, bv_rep, bp_rep = _host_prep(
        x, mask, affine_matrix, Wq, bq, Wkv, bkv, Wp, bp, angle_table, H, W)

    if "nc" not in _CACHE:
        _CACHE["nc"] = build_kernel()
    nc = _CACHE["nc"]

    in_maps = []
    for m in range(N_CORES):
        bs = [BPC * m + j for j in range(BPC)]
        in_maps.append({
            "xT": np.ascontiguousarray(xT[bs]),
            "biasT": np.ascontiguousarray(biasT[bs]),
            "emT": np.ascontiguousarray(emT[[bb % NW for bb in bs]]),
            "wq": Wq, "wk": Wk, "wv": Wv, "wp": Wp,
            "bq": bq2, "bk": bk2, "bv": bv_rep, "bp": bp_rep,
        })

    res = run_bass_kernel_spmd(nc, in_maps, core_ids=list(range(N_CORES)),
                               trace=_profile)
    out = np.empty((B, N, C), np.float32)
    for m in range(N_CORES):
        y = res.results[m]["y"]
        for j in range(BPC):
            out[BPC * m + j] = y[j]
    if _profile:
        return out, res
    return out


if __name__ == "__main__":
    import reference
    inputs = reference.setup_inputs()
    out = kernel(**{k: (np.asarray(v) if hasattr(v, "shape") else v)
                    for k, v in inputs.items()})
    ref = np.asarray(reference.reference(**inputs))
    err = np.abs(out - ref)
    print("max abs err:", err.max(),
          "absmax-rel:", err.max() / np.abs(ref).max())


# revision 20
# speedup vs baseline: 1.2256x; 1.2256x over previous
"""Trainium2 Bass kernel for nn_Attention_10797547782838.

Windowed multi-head attention with per-query angle bias:
  q = (x@Wq+bq) reshaped to heads; k,v = x@Wkv+bkv
  attn = (q*scale) @ k^T * anglebias(q) + mask[b%4]; softmax; @v; proj Wp.

Sharding: batch (16) data-parallel over 8 cores, 2 batches/core.
Device layout: S^T (keys on partitions, queries free). Angle bias and
hd^-0.5 are folded into q at projection time; the window mask is applied
multiplicatively after exp (P = exp(S^T) * exp(mask)^T); softmax
denominators come from ones-column matmuls; normalization is applied to
O^T via a selector-matmul broadcast of 1/rowsum before the output
projection.
"""
import math
import sys

import numpy as np

sys.path.insert(0, "/opt/trn_rl_repo")

import concourse.bass as bass  # noqa: E402
import concourse.bacc as bacc  # noqa: E402
import concourse.tile as tile  # noqa: E402
from concourse import mybir  # noqa: E402
from concourse.bass_utils import run_bass_kernel_spmd  # noqa: E402

F32 = mybir.dt.float32
F32R = mybir.dt.float32r
BF16 = mybir.dt.bfloat16
AF = mybir.ActivationFunctionType
ALU = mybir.AluOpType

B, N, C = 16, 1024, 256
HEADS, HD = 8, 32
NW = 4
N_CORES = 8
BPC = B // N_CORES  # batches per core

_CACHE = {}


def r32(ap):
    return ap.bitcast(F32R)


def build_kernel():
    from contextlib import ExitStack
    nc = bacc.Bacc("TRN2", target_bir_lowering=False, debug=False,
                   num_devices=N_CORES)

    d_xT = nc.dram_tensor("xT", [BPC, C, N], F32R, kind="ExternalInput").ap()
    d_biasT = nc.dram_tensor("biasT", [BPC, C, N], F32, kind="ExternalInput").ap()
    d_emT = nc.dram_tensor("emT", [BPC, N, N], BF16, kind="ExternalInput").ap()
    d_wq = nc.dram_tensor("wq", [C, C], F32R, kind="ExternalInput").ap()
    d_wk = nc.dram_tensor("wk", [C, C], F32R, kind="ExternalInput").ap()
    d_wv = nc.dram_tensor("wv", [C, C], F32R, kind="ExternalInput").ap()
    d_wp = nc.dram_tensor("wp", [C, C], F32R, kind="ExternalInput").ap()
    d_bq = nc.dram_tensor("bq", [128, 2], F32, kind="ExternalInput").ap()
    d_bk = nc.dram_tensor("bk", [128, 2], F32, kind="ExternalInput").ap()
    d_bv = nc.dram_tensor("bv", [128, C], F32, kind="ExternalInput").ap()
    d_bp = nc.dram_tensor("bp", [128, C], F32, kind="ExternalInput").ap()
    d_y = nc.dram_tensor("y", [BPC, N, C], F32, kind="ExternalOutput").ap()

    with tile.TileContext(nc) as tc:
        with ExitStack() as ctx, nc.allow_low_precision(reason="fp32r matmul inputs; accumulation stays fp32 in PSUM"):
            kernel_body(ctx, tc, d_xT, d_biasT, d_emT, d_wq, d_wk, d_wv,
                        d_wp, d_bq, d_bk, d_bv, d_bp, d_y)
    nc.compile()
    return nc


def kernel_body(ctx, tc, d_xT, d_biasT, d_emT, d_wq, d_wk, d_wv, d_wp,
                d_bq, d_bk, d_bv, d_bp, d_y):
    nc = tc.nc

    consts = ctx.enter_context(tc.tile_pool(name="consts", bufs=1))
    xpool = ctx.enter_context(tc.tile_pool(name="xpool", bufs=2))
    qkv = ctx.enter_context(tc.tile_pool(name="qkv", bufs=2))
    empool = ctx.enter_context(tc.tile_pool(name="empool", bufs=2))
    ppool = ctx.enter_context(tc.tile_pool(name="ppool", bufs=4))
    otpool = ctx.enter_context(tc.tile_pool(name="otpool", bufs=2))
    misc = ctx.enter_context(tc.tile_pool(name="misc", bufs=2))
    ypool = ctx.enter_context(tc.tile_pool(name="ypool", bufs=3))
    psS = ctx.enter_context(tc.tile_pool(name="psS", bufs=3, space="PSUM"))
    psO = ctx.enter_context(tc.tile_pool(name="psO", bufs=1, space="PSUM"))

    # ---- constants / weights (once) ----
    w_sb = {}
    for name, dram in (("wq", d_wq), ("wk", d_wk), ("wv", d_wv), ("wp", d_wp)):
        t = consts.tile([128, 2, C], F32R, tag=f"w_{name}")
        nc.sync.dma_start(out=t[:], in_=dram.rearrange("(c p) n -> p c n", p=128))
        w_sb[name] = t
    bq_sb = consts.tile([128, 2], F32, tag="bq")
    nc.sync.dma_start(out=bq_sb[:], in_=d_bq[:])
    bk_sb = consts.tile([128, 2], F32, tag="bk")
    nc.sync.dma_start(out=bk_sb[:], in_=d_bk[:])
    bv_sb = consts.tile([128, C], F32, tag="bv")
    nc.sync.dma_start(out=bv_sb[:], in_=d_bv[:])
    bp_sb = consts.tile([128, C], F32, tag="bp")
    nc.sync.dma_start(out=bp_sb[:], in_=d_bp[:])
    ones_bf = consts.tile([128, 32], BF16, tag="ones_bf")
    nc.vector.memset(ones_bf[:], 1.0)

    n_dve = 0
    n_gps = 0

    v_sb = [qkv.tile([128, C], BF16, tag=f"v{t8}", name=f"v{t8}")
            for t8 in range(8)]

    for b in range(BPC):
        # ---- stage inputs for this batch ----
        xT_sb = xpool.tile([128, 2, N], F32R, tag="xT")
        nc.sync.dma_start(out=xT_sb[:], in_=d_xT[b].rearrange("(c p) n -> p c n", p=128))
        biasT_sb = xpool.tile([128, 2, N], F32, tag="biasT")
        nc.sync.dma_start(out=biasT_sb[:], in_=d_biasT[b].rearrange("(c p) n -> p c n", p=128))
        em_sb = [empool.tile([128, N], BF16, tag=f"em{kc}", name=f"em{kc}") for kc in range(8)]
        for kc in range(8):
            nc.sync.dma_start(out=em_sb[kc][:], in_=d_emT[b, kc * 128:(kc + 1) * 128, :])

        # ---- projections ----
        # qT/kT: [c_out chunk m 128, q 1024]
        qT_sb = qkv.tile([128, 2, N], F32R, tag="qT")
        kT_sb = qkv.tile([128, 2, N], F32R, tag="kT")
        for m in range(2):
            ps_q = psS.tile([128, N], F32, tag="s")
            for qc in range(2):
                for ci in range(2):
                    nc.tensor.matmul(
                        ps_q[:, qc * 512:(qc + 1) * 512],
                        w_sb["wq"][:, ci, m * 128:(m + 1) * 128],
                        xT_sb[:, ci, qc * 512:(qc + 1) * 512],
                        start=(ci == 0), stop=(ci == 1))
            # qT = (psum + bq) * biasT
            nc.vector.scalar_tensor_tensor(
                out=qT_sb[:, m, :], in0=ps_q[:], scalar=bq_sb[:, m:m + 1],
                in1=biasT_sb[:, m, :], op0=ALU.add, op1=ALU.mult)
            ps_k = psS.tile([128, N], F32, tag="s")
            for qc in range(2):
                for ci in range(2):
                    nc.tensor.matmul(
                        ps_k[:, qc * 512:(qc + 1) * 512],
                        w_sb["wk"][:, ci, m * 128:(m + 1) * 128],
                        xT_sb[:, ci, qc * 512:(qc + 1) * 512],
                        start=(ci == 0), stop=(ci == 1))
            nc.vector.tensor_scalar_add(
                out=kT_sb[:, m, :], in0=ps_k[:], scalar1=bk_sb[:, m:m + 1])

        # V: [token chunk 128, c_out 256] bf16
        for t8 in range(8):
            ps_v = psS.tile([128, C], F32, tag="s")
            for ci in range(2):
                nc.tensor.matmul(
                    ps_v[:],
                    xT_sb[:, ci, t8 * 128:(t8 + 1) * 128],
                    w_sb["wv"][:, ci, :],
                    start=(ci == 0), stop=(ci == 1))
            nc.vector.tensor_tensor(
                out=v_sb[t8][:], in0=ps_v[:], in1=bv_sb[:], op=ALU.add)

        # ---- attention ----
        otn_sb = [otpool.tile([128, N], F32R, tag=f"otn{cc}", name=f"otn{cc}") for cc in range(2)]
        for qc in range(2):
            for hg in range(2):  # head group: heads 4hg..4hg+3
                po = psO.tile([128, 512], F32, tag="po")
                pr = psO.tile([128, 512], F32, tag="pr")
                for kc in range(8):
                    p_pair = []
                    for pair in range(2):  # heads 4hg+2pair, +1
                        ps_s = psS.tile([128, 1024], F32, tag="s")
                        for hh in range(2):
                            h = 4 * hg + 2 * pair + hh
                            j = h % 4
                            nc.tensor.matmul(
                                ps_s[:, hh * 512:(hh + 1) * 512],
                                kT_sb[32 * j:32 * (j + 1), hg,
                                          kc * 128:(kc + 1) * 128],
                                qT_sb[32 * j:32 * (j + 1), hg,
                                          qc * 512:(qc + 1) * 512],
                                start=True, stop=True,
                                tile_position=(32 * j, 0))
                        p0 = ppool.tile([128, 1024], BF16, tag="p0")
                        nc.scalar.activation(out=p0[:], in_=ps_s[:], func=AF.Exp)
                        # P = P0 * em (mask), split DVE / gpsimd
                        pt = ppool.tile([128, 1024], BF16, tag="pt")
                        emsl = em_sb[kc][:, qc * 512:(qc + 1) * 512]
                        for hh in range(2):
                            use_dve = n_dve <= 3 * n_gps
                            eng = nc.vector if use_dve else nc.gpsimd
                            if use_dve:
                                n_dve += 1
                            else:
                                n_gps += 1
                            eng.tensor_tensor(
                                out=pt[:, hh * 512:(hh + 1) * 512],
                                in0=p0[:, hh * 512:(hh + 1) * 512],
                                in1=emsl, op=ALU.mult)
                        p_pair.append(pt)
                    for pair in range(2):
                        for hh in range(2):
                            h = 4 * hg + 2 * pair + hh
                            j = h % 4
                            rhs = p_pair[pair][:, hh * 512:(hh + 1) * 512]
                            nc.tensor.matmul(
                                po[32 * j:32 * (j + 1), :],
                                v_sb[kc][:, h * 32:(h + 1) * 32],
                                rhs, start=(kc == 0), stop=(kc == 7),
                                tile_position=(0, 32 * j))
                            nc.tensor.matmul(
                                pr[32 * j:32 * (j + 1), :],
                                ones_bf[:],
                                rhs, start=(kc == 0), stop=(kc == 7),
                                tile_position=(0, 32 * j))
                # pr strip j: all 32 rows = rowsum of head 4hg+j
                rb = misc.tile([128, 512], F32, tag="rb")
                nc.vector.reciprocal(out=rb[:], in_=pr[:])
                nc.vector.tensor_tensor(
                    out=otn_sb[hg][:, qc * 512:(qc + 1) * 512],
                    in0=po[:], in1=rb[:], op=ALU.mult)

        # ---- output projection: Y[q,c] = Onorm^T.T @ Wp + bp ----
        for qt in range(8):
            ps_y = psS.tile([128, C], F32, tag="s")
            for ci in range(2):
                nc.tensor.matmul(
                    ps_y[:],
                    otn_sb[ci][:, qt * 128:(qt + 1) * 128],
                    w_sb["wp"][:, ci, :],
                    start=(ci == 0), stop=(ci == 1))
            y_sb = ypool.tile([128, C], F32, tag="y")
            nc.vector.tensor_tensor(out=y_sb[:], in0=ps_y[:], in1=bp_sb[:],
                                    op=ALU.add)
            nc.sync.dma_start(out=d_y[b, qt * 128:(qt + 1) * 128, :], in_=y_sb[:])


def _host_prep(x, mask, affine_matrix, Wq, bq, Wkv, bkv, Wp, bp,
               angle_table, H, W):
    B_, N_, C_ = x.shape
    heads = angle_table.shape[1]
    hd = C_ // heads
    scale = np.float32(hd ** -0.5)
    H = int(H); W = int(W)

    gy, gx = np.meshgrid(np.arange(H, dtype=np.float32),
                         np.arange(W, dtype=np.float32), indexing="ij")
    coords = np.stack([gx.reshape(-1), gy.reshape(-1)], -1).astype(np.float32)
    center = np.array([W / 2.0, H / 2.0], np.float32)
    ego = np.einsum("bij,j->bi", affine_matrix[:, :2, :2], center) \
        + affine_matrix[:, :2, 2]
    rel = coords[None, :, :] - ego[:, None, :]
    ang = np.arctan2(rel[..., 1], rel[..., 0]).astype(np.float32)
    bins = (((ang + np.float32(math.pi)) / np.float32(2.0 * math.pi))
            * (angle_table.shape[0] - 1)).astype(np.int32)
    sig = (1.0 / (1.0 + np.exp(-angle_table[bins]))).astype(np.float32)
    bias = (1.0 + sig).astype(np.float32)                      # (B,N,h)

    biasT = np.repeat(bias.transpose(0, 2, 1) * scale, hd, axis=1)  # (B,C,N)
    biasT = np.ascontiguousarray(biasT, dtype=np.float32)
    xT = np.ascontiguousarray(x.transpose(0, 2, 1), dtype=np.float32)
    import ml_dtypes
    emT = np.ascontiguousarray(
        np.exp(mask).transpose(0, 2, 1).astype(ml_dtypes.bfloat16))  # [k,q]

    Wk = np.ascontiguousarray(Wkv[:, :C_], dtype=np.float32)
    Wv = np.ascontiguousarray(Wkv[:, C_:], dtype=np.float32)
    bq2 = np.ascontiguousarray(bq.reshape(2, 128).T, dtype=np.float32)
    bk2 = np.ascontiguousarray(bkv[:C_].reshape(2, 128).T, dtype=np.float32)
    bv_rep = np.ascontiguousarray(
        np.broadcast_to(bkv[C_:], (128, C_)), dtype=np.float32)
    bp_rep = np.ascontiguousarray(
        np.broadcast_to(bp, (128, C_)), dtype=np.float32)
    return xT, biasT, emT, Wk, Wv, bq2, bk2, bv_rep, bp_rep


def _ensure_ntff_hook():
    import types
    try:
        from antenv import axon_hooks  # noqa: F401
        return
    except ImportError:
        pass
    import antenv
    mod = types.ModuleType("antenv.axon_hooks")
    _h = {"hook": None}
    mod.get_axon_ntff_profile_hook = lambda: _h["hook"]
    mod.set_axon_ntff_profile_hook = lambda hook: _h.__setitem__("hook", hook)
    sys.modules["antenv.axon_hooks"] = mod
    antenv.axon_hooks = mod
    try:
        sys.path.insert(0, "/root/.axon_site/trn_agent_boot")
        import trn_boot
        hook = trn_boot._ntff_profile_via_ctypes("/opt/axon/libaxon_pjrt.so")
        if hook is not None:
            mod.set_axon_ntff_profile_hook(hook)
    except Exception as e:
        print("ntff hook setup failed:", repr(e))


def kernel(x, mask, affine_matrix, Wq, bq, Wkv, bkv, Wp, bp,
           angle_table, H, W, _profile=False):
    if _profile:
        _ensure_ntff_hook()
    x = np.asarray(x, np.float32)
    mask = np.asarray(mask, np.float32)
    affine_matrix = np.asarray(affine_matrix, np.float32)
    Wq = np.asarray(Wq, np.float32); bq = np.asarray(bq, np.float32)
    Wkv = np.asarray(Wkv, np.float32); bkv = np.asarray(bkv, np.float32)
    Wp = np.asarray(Wp, np.float32); bp = np.asarray(bp, np.float32)
    angle_table = np.asarray(angle_table, np.float32)

    xT, biasT, emT, Wk, Wv, bq2, bk2, bv_rep, bp_rep = _host_prep(
        x, mask, affine_matrix, Wq, bq, Wkv, bkv, Wp, bp, angle_table, H, W)

    if "nc" not in _CACHE:
        _CACHE["nc"] = build_kernel()
    nc = _CACHE["nc"]

    in_maps = []
    for m in range(N_CORES):
        bs = [BPC * m + j for j in range(BPC)]
        in_maps.append({
            "xT": np.ascontiguousarray(xT[bs]),
            "biasT": np.ascontiguousarray(biasT[bs]),
            "emT": np.ascontiguousarray(emT[[bb % NW for bb in bs]]),
            "wq": Wq, "wk": Wk, "wv": Wv, "wp": Wp,
            "bq": bq2, "bk": bk2, "bv": bv_rep, "bp": bp_rep,
        })

    res = run_bass_kernel_spmd(nc, in_maps, core_ids=list(range(N_CORES)),
                               trace=_profile)
    out = np.empty((B, N, C), np.float32)
    for m in range(N_CORES):
        y = res.results[m]["y"]
        for j in range(BPC):
            out[BPC * m + j] = y[j]
    if _profile:
        return out, res
    return out


if __name__ == "__main__":
    import reference
    inputs = reference.setup_inputs()
    out = kernel(**{k: (np.asarray(v) if hasattr(v, "shape") else v)
                    for k, v in inputs.items()})
    ref = np.asarray(reference.reference(**inputs))
    err = np.abs(out - ref)
    print("max abs err:", err.max(),
          "absmax-rel:", err.max() / np.abs(ref).max())
